# revision 27
# baseline (speedup 1.0000x reference)
"""Trainium2 Bass kernel for nn_AttnDecoder (protein conv encoder + GO attention).

Strategy: data-parallel over batch — 32 samples -> 4 per NeuronCore x 8 cores,
all parameters replicated.  The GO embedding gather and the tiny aa-embedding
gather are done host-side as part of input distribution.

v3 math restructure (per sample), all device arithmetic in fp16 (same PE
rate as bf16 but 8x finer mantissa -> rel err ~2e-3 vs bf16's 1.5e-2):
  - conv1 as a dense K=80 fp16 matmul (host im2col), as before.
  - conv2 via Winograd F(2,3): the 15-tap conv = 5 blocks of 3 taps; each
    block needs 4 products per output PAIR instead of 6 -> 320 matmuls of
    N=505 per sample instead of 480 (1.5x less PE time).  The four Winograd
    coordinates tau accumulate in four PSUM banks over (5 blocks x 4
    in-channel chunks); the A^T output transform runs on ACT+DVE:
      x2[2m]   = relu(M0 + M1 + M2),  x2[2m+1] = relu(M1 - M2 - M3)
  - the B^T input transform is 3 full-length arrays per in-channel chunk,
    produced on DVE with contiguous reads/writes (2x packed mode):
      E[p] = x1[p]-x1[p+2], S[p] = x1[p]+x1[p+1], T[p] = x1[p+1]-x1[p]
    The matmul rhs reads them at stride 2 (phase = block parity), which is
    free on the PE.  GpSimd is deliberately idle: its tensor ops run at
    ~40G elem/s and poison the PE's SBUF bandwidth.
  - x2 is stored parity-major ([c, parity, pair]) so Winograd combines write
    contiguously; energies/ctx split each 252-section window into the two
    parities (a softmax-invariant permutation of the sections).
  - attention: v = go @ attn_w once per core (emitted under sample 0's
    conv2); energies via 32 M=1 fp16 matmuls into one PSUM bank, lagged one
    conv2 group behind the combines that feed them (the last batch slides
    into the next sample); softmax on-device; context via a batched DVE
    multiply (attn broadcast across sections) + per-section reduce; output
    unpermuted on host.  attn_b is softmax-invariant and dropped.
"""

import numpy as np

import concourse.bass as bass
import concourse.mybir as mybir
import concourse.tile as tile
from concourse.bass_utils import run_bass_kernel_spmd

# ---- problem constants (must match the reference) ----
B, L = 32, 2048
NCORES = 8
BPC = B // NCORES          # samples per core
AA_VOCAB, AA_EMB = 26, 5
C = 256                    # conv2 out channels
C2 = 512                   # conv1 out channels
S = 8                      # section size
CS = C * S                 # 2048
GO = 256                   # go embedding dim
KS = 15                    # conv kernel size
L1 = L - KS + 1            # 2034, conv1 output length
P2 = L1 - KS + 1           # 2020, conv2 output length
NSEC = P2 // S             # 252 sections
HSEC = NSEC // 2           # 126 per parity
HEAD = (P2 % S) // 2       # 2, head trim of the section view
KE = 80                    # conv1 im2col rows: 15 taps x 5 emb = 75 -> pad 80
KC1 = 4                    # conv2 in-channel chunks: 512 -> 4x128
N1_TILES = (0, 512, 1024, 1536)
NP = P2 // 2               # 1010 output pairs
HNP = NP // 2              # 505 pairs per half
ESTW = 1024                # est array width per half (1022/1023 used)
# est array ids (full-length, both parities interleaved)
EA, SA, TA = range(3)

F32 = mybir.dt.float32
F16 = mybir.dt.float16
BF16 = mybir.dt.bfloat16
TRACE = False
LAST_RESULT = None

_NC_CACHE = {}


def _wino_rhs(tau, j):
    """(array id, column offset) for the rhs of block j, coordinate tau.
    The rhs reads the full-length array at stride 2 from this offset."""
    return ((EA, SA, TA, EA)[tau], 3 * j + (0 if tau == 0 else 1))


def _build():
    nc = bass.Bass()
    im2_d = nc.dram_tensor("im2", [BPC, KE, L1], F16, kind="ExternalInput")
    w1e_d = nc.dram_tensor("w1e", [KE, C2], F16, kind="ExternalInput")
    b1_d = nc.dram_tensor("b1", [128, 4], F32, kind="ExternalInput")
    # gw[p, mt, tau, kc, j, o] = Gw[128*mt+o, 128*kc+p, j, tau]
    gw_d = nc.dram_tensor("gw", [128, 2, 4, KC1, 5, 128], F16,
                          kind="ExternalInput")
    b2_d = nc.dram_tensor("b2", [128, 2], F32, kind="ExternalInput")
    goT_d = nc.dram_tensor("goT", [GO, BPC], F16, kind="ExternalInput")
    attnw_d = nc.dram_tensor("attnw", [GO, CS], F16, kind="ExternalInput")
    out_d = nc.dram_tensor("out", [BPC, 128, 2 * S], F32, kind="ExternalOutput")

    RELU = mybir.ActivationFunctionType.Relu

    with (
        tile.TileContext(nc) as tc,
        tc.tile_pool(name="singles", bufs=1) as singles,
        tc.tile_pool(name="persamp", bufs=2) as persamp,
        tc.tile_pool(name="big", bufs=2) as big,
        tc.tile_pool(name="estp", bufs=2) as estp,
        tc.tile_pool(name="ctmp", bufs=2) as ctmp,
        tc.tile_pool(name="mm", bufs=6, space="PSUM") as mmpool,
        tc.tile_pool(name="spsum", bufs=1, space="PSUM") as spsum,
        tc.tile_pool(name="dram", bufs=2, space="DRAM") as dpool,
    ):
        # ---- resident weights / constants ----
        # DMA queue plan (two HWDGE queues: SP via nc.sync, Act via
        # nc.scalar).  conv1 needs w1+im2(b0) first; conv2's first group
        # needs gw[mt0, tau0] first, then tau1.. in matmul order.
        w1sb = singles.tile([KE, C2], F16)
        nc.scalar.dma_start(w1sb, w1e_d[:, :])
        rhs1_s0 = big.tile([KE, L1], F16, tag="rhs1")
        nc.scalar.dma_start(rhs1_s0[:, :1024], im2_d[0, :, :1024])
        nc.sync.dma_start(rhs1_s0[:, 1024:], im2_d[0, :, 1024:])
        b1sb = singles.tile([128, 4], F32)
        nc.sync.dma_start(b1sb, b1_d[:, :])
        b2sb = singles.tile([128, 2], F32)
        nc.sync.dma_start(b2sb, b2_d[:, :])
        gwsb = singles.tile([128, 2, 4, KC1, 5, 128], F16)
        for mt in range(2):
            for tau in range(4):
                q = nc.scalar if (2 * mt + tau) % 2 == 0 else nc.sync
                q.dma_start(gwsb[:, mt, tau], gw_d[:, mt, tau])
        onesb = singles.tile([1, 128], F16)
        nc.vector.memset(onesb, 1.0)

        # V = go_sel @ attn_w is emitted inside sample 0, after its conv2.
        vdram = dpool.tile([BPC, 128, 2 * S], F16)

        def emit_v():
            gosb = singles.tile([128, 2, BPC], F16)
            nc.sync.dma_start(gosb, goT_d.rearrange("(c p) b -> p c b", p=128))
            awsb = singles.tile([128, 2, CS], F16)
            nc.sync.dma_start(awsb, attnw_d.rearrange("(c p) n -> p c n", p=128))
            # vsb free layout [c, p, s] == flat V index q = c*1024 + p*8 + s
            vsb = singles.tile([BPC, 2, 128, S], F16)
            for n in range(4):
                vps = mmpool.tile([128, 512], F32, tag="mm512")
                for c in range(2):
                    nc.tensor.matmul(
                        vps[:BPC, :],
                        gosb[:, c, :],
                        awsb[:, c, 512 * n : 512 * (n + 1)],
                        start=(c == 0),
                        stop=(c == 1),
                    )
                nc.scalar.copy(
                    vsb[:, n // 2, 64 * (n % 2) : 64 * (n % 2) + 64, :],
                    vps[:BPC, :],
                )
            with nc.allow_non_contiguous_dma(reason="permute V to channel-major"):
                for c in range(2):
                    nc.sync.dma_start(
                        vdram[:, :, S * c : S * (c + 1)], vsb[:, c]
                    )

        prev = None
        deferred_en = None
        for b in range(BPC):
            # conv1 tiles 0-1 cover x1 cols [0, 1024) -> enough for est half 0
            rhs1, x1 = _sample_conv1(
                nc, b, im2_d, big, mmpool, w1sb, b1sb,
                rhs1_s0 if b == 0 else None, n_tiles=N1_TILES[:2],
            )
            # previous sample's last energies batch: its x2 combines finish
            # ~1.3us after that sample's last matmul; conv1 above hides it
            if deferred_en is not None:
                deferred_en()
                deferred_en = None
            est0 = _emit_est(nc, x1, estp, half=0)
            est_h = [est0, None]
            _sample_conv1(
                nc, b, im2_d, big, mmpool, w1sb, b1sb,
                rhs1=rhs1, x1=x1, n_tiles=N1_TILES[2:3],
            )

            if prev is not None:
                _attn_tail(nc, *prev, out_d, persamp, onesb, spsum)

            vmat = None
            if b > 0:
                vmat = persamp.tile([128, 2, S], F16, tag="vmat")
                nc.sync.dma_start(vmat, vdram[b])

            def emit_mid(bb=b, r=rhs1, x=x1, eh=est_h):
                _sample_conv1(
                    nc, bb, im2_d, big, mmpool, w1sb, b1sb,
                    rhs1=r, x1=x, n_tiles=N1_TILES[3:],
                )
                eh[1] = _emit_est(nc, x, estp, half=1)

            x2 = big.tile([128, 2, 2, NP], F16, tag="x2")
            eng = spsum.tile([1, NSEC], F32, tag="eng")
            groups = ((0, 0), (1, 0), (0, 1), (1, 1))
            for gi, (mt, h) in enumerate(groups):
                _conv2_group(
                    nc, x2, est_h, gwsb, b2sb, mmpool, ctmp, mt, h,
                    emit_mid if gi == 0 else None,
                )
                if b == 0 and gi == 0:
                    # V matmuls + DRAM roundtrip hide under groups 1-3
                    emit_v()
                    vmat = persamp.tile([128, 2, S], F16, tag="vmat")
                    nc.sync.dma_start(vmat, vdram[b])
                if gi > 0:
                    # lag energies one group behind conv2 so they never
                    # wait on just-finished combines
                    pm, ph_ = groups[gi - 1]
                    _emit_energies(nc, eng, vmat, x2, pm, ph_,
                                   first=(gi == 1), last=False)

            def _defer(e=eng, v=vmat, xx=x2, m=groups[3][0], hh=groups[3][1]):
                _emit_energies(nc, e, v, xx, m, hh,
                               first=False, last=True)
            deferred_en = _defer
            prev = (b, eng, x2)
        if deferred_en is not None:
            deferred_en()
        _attn_tail(nc, *prev, out_d, persamp, onesb, spsum)
    return nc


def _sample_conv1(nc, b, im2_d, big, mmpool, w1sb, b1sb, rhs1=None,
                  x1=None, n_tiles=N1_TILES):
    RELU = mybir.ActivationFunctionType.Relu

    if rhs1 is None:
        # host-precomputed im2col of the embedded sequence
        rhs1 = big.tile([KE, L1], F16, tag="rhs1")
        nc.sync.dma_start(rhs1, im2_d[b])

    # conv1 + bias + relu -> x1 [512ch, 2034] fp16 (K=80 single pass).
    # Matmuls n0-major (reuses the loaded weights across column tiles);
    # evacs emitted m-major so est's E[kc] production starts after 2 evacs.
    # All evacs on ACT (DVE is the est/combine critical path).
    if x1 is None:
        x1 = big.tile([128, 4, L1], F16, tag="x1")
    pss = {}
    for n0 in n_tiles:
        for m in range(4):
            nn = min(512, L1 - n0)
            ps = mmpool.tile([128, 512], F32, tag="mm512", name=f"c1ps{m}_{n0}")
            nc.tensor.matmul(
                ps[:, :nn],
                w1sb[:, 128 * m : 128 * (m + 1)],
                rhs1[:, n0 : n0 + nn],
                start=True,
                stop=True,
            )
            pss[(m, n0)] = ps
    for m in range(4):
        for n0 in n_tiles:
            nn = min(512, L1 - n0)
            nc.scalar.activation(
                out=x1[:, m, n0 : n0 + nn],
                in_=pss[(m, n0)][:, :nn],
                func=RELU,
                bias=b1sb[:, m : m + 1],
                scale=1.0,
            )
    return rhs1, x1


def _emit_est(nc, x1, estp, half):
    """B^T input transform: 3 full-length arrays per in-channel chunk,
    all reads/writes contiguous (the matmul rhs reads them at stride 2):
      E[p] = x1[p] - x1[p+2],  S[p] = x1[p] + x1[p+1],  T[p] = x1[p+1] - x1[p]
    est[:, kc, X, p'] = X[1010*half + p'].  DVE takes E,S; GpSimd takes T."""
    est = estp.tile([128, KC1, 3, ESTW], F16, tag="est")
    c0 = 2 * HNP * half
    # matmuls read p' <= 1021; half 0 may only touch x1 cols [0, 1024)
    ne = ns = 1022
    SUB = mybir.AluOpType.subtract
    ADD = mybir.AluOpType.add
    # E first (consumed by tau 0, the group's first 20 matmuls), S second;
    # T on the (slow) GpSimd runs concurrently and is consumed last
    # (tau order 0,1,3,2 in _conv2_group).
    for kc in range(KC1):
        nc.vector.tensor_tensor(
            est[:, kc, EA, :ne],
            x1[:, kc, c0 : c0 + ne], x1[:, kc, c0 + 2 : c0 + 2 + ne], SUB,
        )
    for kc in range(KC1):
        nc.vector.tensor_tensor(
            est[:, kc, SA, :ns],
            x1[:, kc, c0 : c0 + ns], x1[:, kc, c0 + 1 : c0 + 1 + ns], ADD,
        )
    for kc in range(KC1):
        nc.vector.tensor_tensor(
            est[:, kc, TA, :ns],
            x1[:, kc, c0 + 1 : c0 + 1 + ns], x1[:, kc, c0 : c0 + ns], SUB,
        )
    return est


def _conv2_group(nc, x2, est_h, gwsb, b2sb, mmpool, ctmp, mt, h, emit_mid):
    """One Winograd group: out-channel tile mt, pair half h.  80 matmuls
    (4 tau banks x 5 blocks x 4 kc), then the A^T combine:
      x2[:, mt, 0, half] = relu(M0 + M1 + M2 + b2)
      x2[:, mt, 1, half] = relu(M1 - M2 - M3 + b2)"""
    RELU = mybir.ActivationFunctionType.Relu
    ADD = mybir.AluOpType.add
    SUB = mybir.AluOpType.subtract

    ps = [
        mmpool.tile([128, 512], F32, tag="mm512", name=f"wps{mt}{h}{t}")
        for t in range(4)
    ]
    for tau in (0, 1, 3, 2):
        for kc in range(KC1):
            for j in range(5):
                arr, off = _wino_rhs(tau, j)
                rhs = est_h[h][:, kc, arr, off : off + 2 * HNP].rearrange(
                    "p (m two) -> p two m", two=2
                )[:, 0]
                nc.tensor.matmul(
                    ps[tau][:, :HNP],
                    gwsb[:, mt, tau, kc, j, :],
                    rhs,
                    start=(kc == 0 and j == 0),
                    stop=(kc == KC1 - 1 and j == 4),
                )
                if emit_mid is not None and tau == 0 and kc == 2 and j == 0:
                    emit_mid()
                    emit_mid = None

    n0 = HNP * h
    c0t = ctmp.tile([128, HNP], F16, tag="c0")
    s1t = ctmp.tile([128, HNP], F16, tag="s1")
    ept = ctmp.tile([128, HNP], F16, tag="ep")
    c3t = ctmp.tile([128, HNP], F16, tag="c3")
    s2t = ctmp.tile([128, HNP], F16, tag="s2")
    opt = ctmp.tile([128, HNP], F16, tag="op")
    nc.scalar.copy(c0t, ps[0][:, :HNP])
    nc.vector.tensor_tensor(s1t, ps[1][:, :HNP], c0t, ADD)
    nc.scalar.copy(c3t, ps[3][:, :HNP])
    nc.vector.tensor_tensor(ept, ps[2][:, :HNP], s1t, ADD)
    nc.scalar.activation(
        out=x2[:, mt, 0, n0 : n0 + HNP], in_=ept, func=RELU,
        bias=b2sb[:, mt : mt + 1], scale=1.0,
    )
    nc.vector.tensor_tensor(s2t, ps[2][:, :HNP], c3t, ADD)
    nc.vector.tensor_tensor(opt, ps[1][:, :HNP], s2t, SUB)
    nc.scalar.activation(
        out=x2[:, mt, 1, n0 : n0 + HNP], in_=opt, func=RELU,
        bias=b2sb[:, mt : mt + 1], scale=1.0,
    )


def _emit_energies(nc, eng, vmat, x2, mt, h, first, last):
    """8 M=1 matmuls: sections s in [4h, 4h+4), both parities, channel tile
    mt.  eng layout: [even sections 0:126 | odd sections 126:252] per s...
    actually eng[0, par*126 + m'] accumulates sum over (c, s) with the
    parity-m' section permutation (softmax-invariant)."""
    idx = 0
    for s in range(4 * h, 4 * h + 4):
        for par in range(2):
            nc.tensor.matmul(
                eng[:, HSEC * par : HSEC * par + HSEC],
                vmat[:, mt, s : s + 1],
                x2[:, mt, par, 1 + HSEC * s : 1 + HSEC * s + HSEC],
                start=(first and idx == 0),
                stop=(last and idx == 7),
                skip_group_check=True,
            )
            idx += 1


def _attn_tail(nc, b, eng, x2, out_d, persamp, onesb, spsum):
    """Softmax + attn broadcast + context for sample b (emitted after the
    NEXT sample's conv1 so the broadcast matmul doesn't stall the PE)."""
    EXP = mybir.ActivationFunctionType.Exp
    AX = mybir.AxisListType.X

    # no max-subtraction: |energy| <= ~20 for this model's scale
    expd = persamp.tile([1, NSEC], F32)
    nc.scalar.activation(out=expd, in_=eng, func=EXP, scale=1.0)
    ssum = persamp.tile([1, 1], F32)
    nc.vector.reduce_sum(ssum, expd, axis=AX)
    rsum = persamp.tile([1, 1], F32)
    nc.vector.reciprocal(rsum, ssum)
    attn_t = persamp.tile([1, NSEC], F16)
    nc.vector.tensor_scalar_mul(attn_t, expd, rsum)

    # broadcast attn over 128 partitions with a K=1 ones matmul
    aps = spsum.tile([128, NSEC], F32, tag="abc")
    nc.tensor.matmul(aps, onesb, attn_t, start=True, stop=True)
    attnb = persamp.tile([128, NSEC], F16)
    nc.scalar.copy(attnb, aps)
    attnb2 = attnb.rearrange("p (two m) -> p two m", two=2)

    # ctx[(c,s)] = sum_par sum_m attn[par,m] * x2[c, par, 1+126s+m]
    # batched: one multiply over all 8 sections (attn broadcast across s),
    # one per-section reduce; then even+odd parity add.
    ctx_p = persamp.tile([128, 2, 2, S], F32)
    for c in range(2):
        for par in range(2):
            prod = persamp.tile([128, S, HSEC], F16, tag="prod")
            nc.vector.tensor_tensor(
                prod,
                x2[:, c, par, 1 : 1 + S * HSEC].rearrange(
                    "p (s m) -> p s m", s=S
                ),
                attnb2[:, par : par + 1, :].to_broadcast((128, S, HSEC)),
                mybir.AluOpType.mult,
            )
            nc.vector.reduce_sum(ctx_p[:, c, par], prod, axis=mybir.AxisListType.X)
    ctx_t = persamp.tile([128, 2, S], F32)
    nc.vector.tensor_tensor(
        ctx_t, ctx_p[:, :, 0], ctx_p[:, :, 1], mybir.AluOpType.add
    )
    nc.scalar.dma_start(out_d[b], ctx_t)


def _hoist_excess_waits(nc, cap=1):
    """Walrus codegen fits only one sem-wait slot on a Matmult (the LDWEIGHTS
    struct), but Tile attaches one wait per producer processor.  Hoist the
    excess waits onto standalone EventSemaphore instructions inserted just
    before the offender on the same engine queue — queues execute in order,
    so this is semantically identical."""
    import json as _json

    bir = _json.loads(nc.to_json_bytes())
    ctr = [0]

    def fix_block(b):
        insts = b.get("instructions")
        if insts:
            new = []
            for ins in insts:
                si = ins.get("sync_info")
                waits = (si or {}).get("on_wait") or []
                if len(waits) > cap:
                    keep = waits[len(waits) - cap :] if cap else []
                    for w in waits[: len(waits) - cap]:
                        ctr[0] += 1
                        new.append(
                            {
                                "debug": ins.get("debug"),
                                "engine": ins["engine"],
                                "ins": [],
                                "name": f"hoistw-{ctr[0]}",
                                "opcode": "EventSemaphore",
                                "outs": [],
                                "sync_info": {"on_update": [], "on_wait": [w]},
                            }
                        )
                    si["on_wait"] = keep
                new.append(ins)
            b["instructions"] = new
        for sb in b.get("blocks") or []:
            fix_block(sb)

    for fnc in bir["functions"]:
        for b in fnc["blocks"]:
            fix_block(b)
    patched = _json.dumps(bir).encode()
    nc.to_json_bytes = lambda: patched
    return ctr[0]


def get_nc():
    if "v3" not in _NC_CACHE:
        nc = _build()
        n = _hoist_excess_waits(nc)
        print(f"hoisted {n} excess matmul waits", flush=True)
        _NC_CACHE["v3"] = nc
    return _NC_CACHE["v3"]


def prep_in_maps(
    input_seq,
    input_go_term,
    aa_emb,
    conv1_w,
    conv1_b,
    conv2_w,
    conv2_b,
    go_table,
    attn_w,
    attn_b,
):
    seq = np.asarray(input_seq).astype(np.int64)
    got = np.asarray(input_go_term).astype(np.int64)
    aa = np.asarray(aa_emb).astype(np.float32)
    w1 = np.asarray(conv1_w).astype(np.float32)
    b1 = np.asarray(conv1_b).astype(np.float32)
    w2 = np.asarray(conv2_w).astype(np.float32)
    b2 = np.asarray(conv2_b).astype(np.float32)
    gt = np.asarray(go_table).astype(np.float32)
    aw = np.asarray(attn_w).astype(np.float32)
    # attn_b shifts all of a sample's energies by one constant -> softmax
    # invariant, so it never reaches the device.

    f16 = np.float16

    # conv1 weights as [(k,e), o] with vocab folded via the host-side gather
    w1e = np.zeros((KE, C2), np.float32)
    w1e[: KS * AA_EMB] = w1.transpose(2, 1, 0).reshape(KS * AA_EMB, C2)
    w1e = w1e.astype(f16)

    # Winograd G-transform of conv2 weights:
    # Gw[c, i, j, tau]: (w0, (w0+w1+w2)/2, (w0-w1+w2)/2, w2) per 3-tap block
    wb = w2.reshape(C, C2, 5, 3)
    gwm = np.stack(
        [
            wb[..., 0],
            0.5 * (wb[..., 0] + wb[..., 1] + wb[..., 2]),
            0.5 * (wb[..., 0] - wb[..., 1] + wb[..., 2]),
            wb[..., 2],
        ],
        axis=-1,
    )  # [c, i, j, tau]
    # gw[p, mt, tau, kc, j, o] = Gw[128*mt+o, 128*kc+p, j, tau]
    gwt = gwm.reshape(2, 128, KC1, 128, 5, 4)  # [mt, o, kc, p, j, tau]
    gwt = np.ascontiguousarray(gwt.transpose(3, 0, 5, 2, 4, 1)).astype(f16)

    awh = aw.astype(f16)
    b1t = np.ascontiguousarray(b1.reshape(4, 128).T)
    b2t = np.ascontiguousarray(b2.reshape(2, 128).T)

    # host im2col of the embedded sequence: [B, 80, L1] fp16
    xe = aa.astype(f16).astype(np.float32)[seq]        # [B, L, 5]
    xe = np.ascontiguousarray(xe.transpose(0, 2, 1))   # [B, 5, L]
    win = np.lib.stride_tricks.sliding_window_view(xe, L1, axis=2)  # [B,5,15,L1]
    im2 = np.zeros((B, KE, L1), np.float32)
    im2[:, : KS * AA_EMB] = win.transpose(0, 2, 1, 3).reshape(B, KS * AA_EMB, L1)
    im2 = im2.astype(f16)

    go_sel = gt[got]  # [B, 256]

    in_maps = []
    for core in range(NCORES):
        sl = slice(core * BPC, (core + 1) * BPC)
        in_maps.append(
            {
                "im2": np.ascontiguousarray(im2[sl]),
                "w1e": w1e,
                "b1": b1t,
                "gw": gwt,
                "b2": b2t,
                "goT": np.ascontiguousarray(go_sel[sl].T).astype(f16),
                "attnw": awh,
            }
        )
    return in_maps


def kernel(**inputs):
    global LAST_RESULT
    nc = get_nc()
    in_maps = prep_in_maps(**inputs)
    res = run_bass_kernel_spmd(
        nc, in_maps, core_ids=list(range(NCORES)), trace=TRACE
    )
    LAST_RESULT = res
    dev = np.concatenate([r["out"] for r in res.results], axis=0)
    # dev[b, p, c*S+s] -> out[b, (c*128+p)*S+s]
    dev = dev.reshape(B, 128, 2, S).transpose(0, 2, 1, 3).reshape(B, CS)
    return np.ascontiguousarray(dev)
